# revision 55
# baseline (speedup 1.0000x reference)
"""DiagBlockAttention Trainium2 kernel.

Full module: qkv = x @ w_qkv.T; block-diagonal attention over 16-token
groups (4x4 tiles of the (8, 512) token grid); out = attn_out @ w_out.T + b_out.

Sharding: data-parallel over batch -- batch element i runs on NeuronCore i
(no collectives).  All heavy matmuls run in bf16 on the TensorEngine with
fp32 PSUM accumulation (rel err vs fp32 oracle ~4e-3).

Per-core dataflow (x_b: [4096, 512] fp32):
  1. x -> SBUF, transpose to xT [512_fi, 4096_tok] via regular bf16
     identity matmuls (transpose-mode doesn't count as PE-busy for the HAM
     clock gate, regular matmuls do).  The 4x4-block token permutation that
     makes each 16-token attention group contiguous is folded into the
     free-dim access pattern of the PSUM->SBUF eviction.
  2. GEMM1a: qkT = W_qk-chunks.T @ xT -> feature-major q/k [1024_fo, tok].
     GEMM1b: v = xT-chunks.T @ W_v -> token-major v [tok, 512], stored with
     a ones-column per head (fused softmax denominator).
  3. Attention per (128-token tile, head): S^T = k^T.T @ q^T into PSUM
     [k, q], plus a rank-8 mask matmul (indicator rows valued 16) that adds
     +256 to same-group score entries; P = exp(S/8 - 32) on ScalarE (off-
     group entries -> e^-32 ~ 0, so the dense softmax equals the group-
     restricted softmax; the uniform in-group offset cancels).  One matmul
     per head computes [O' | denom] = P.T @ [v | 1]; normalize+evict via a
     broadcasted reciprocal multiply on VectorE.
  4. O -> O^T via regular bf16 identity matmuls; GEMM2 (pipelined one
     iteration behind): final = O^T-chunks.T @ W_out + b_out; stores
     un-permute back to natural token order.

Hardware notes baked into the structure (found by bisection on trn2):
  - a PSUM accumulation group whose matmuls interleave with another open
    group, or certain K=64 single-matmul groups at base_partition 64,
    fault the exec unit -> groups are kept strictly sequential and closed
    by a base-0 matmul (the mask matmul closes each S group);
  - LDWEIGHTS does not overlap its paired MATMUL on this toolchain
    (walrus --enable-ldw-opt is broken), so per-matmul cost ~ LDW + N.
"""

import os
import sys
from contextlib import ExitStack

sys.path.insert(0, "/opt/trn_rl_repo")

import ml_dtypes
import numpy as np

import concourse.bass as bass
import concourse.mybir as mybir
import concourse.tile as tile
from concourse import bacc
from concourse.bass_utils import run_bass_kernel_spmd


def _ensure_ntff_hook():
    """This image's antenv lacks axon_hooks; synthesize it so trace=True
    (NTFF profiling) works through run_bass_kernel_spmd."""
    import types

    try:
        from antenv import axon_hooks  # noqa: F401
        return
    except ImportError:
        pass
    try:
        import antenv
        from trn_agent_boot.trn_boot import _ntff_profile_via_ctypes

        mod = types.ModuleType("antenv.axon_hooks")
        _hook = [None]
        mod.set_axon_ntff_profile_hook = lambda h: _hook.__setitem__(0, h)
        mod.get_axon_ntff_profile_hook = lambda: _hook[0]
        sys.modules["antenv.axon_hooks"] = mod
        antenv.axon_hooks = mod
        mod.set_axon_ntff_profile_hook(
            _ntff_profile_via_ctypes("/opt/axon/libaxon_pjrt.so"))
    except Exception as e:  # pragma: no cover
        print(f"ntff hook shim failed ({e}); tracing disabled", file=sys.stderr)


_ensure_ntff_hook()

if os.environ.get("KERNEL_LDWOPT") == "1":
    from concourse import bass_utils as _bu

    _orig_run_command = _bu.run_command

    def _run_command_ldwopt(cmd, *a, **kw):
        cmd = ["--enable-ldw-opt=true" if c == "--enable-ldw-opt=false" else c
               for c in cmd]
        return _orig_run_command(cmd, *a, **kw)

    _bu.run_command = _run_command_ldwopt

F32 = mybir.dt.float32
F32R = mybir.dt.float32r
BF16 = mybir.dt.bfloat16

N_CORES = 8
NT = 4096          # tokens per core
DIM = 512          # model dim
INNER = 512        # heads * dim_head
HEADS = 8
DH = 64            # dim head
NSUP = NT // 512   # 512-token supertiles
P = 128

SCALE = DH ** -0.5

# Matmul compute dtype for the three big GEMMs: "fp32r" | "bf16" | "fp32"
MM_MODE = os.environ.get("KERNEL_MM_MODE", "bf16")
DT_MM = {"fp32r": F32R, "bf16": BF16, "fp32": F32}[MM_MODE]
NP_MM = {"fp32r": np.float32, "bf16": ml_dtypes.bfloat16,
         "fp32": np.float32}[MM_MODE]
O_DT = BF16


def build_kernel():
    nc = bacc.Bacc("TRN2", target_bir_lowering=False, debug=False)

    x = nc.dram_tensor("x", [NT, DIM], BF16, kind="ExternalInput").ap()
    wqkvT = nc.dram_tensor(
        "wqkvT", [DIM, 3 * INNER], DT_MM, kind="ExternalInput").ap()
    woutT = nc.dram_tensor("woutT", [INNER, DIM], DT_MM, kind="ExternalInput").ap()
    biasb = nc.dram_tensor("biasb", [P, DIM], F32, kind="ExternalInput").ap()
    identc = nc.dram_tensor("identc", [P, P], F32, kind="ExternalInput").ap()
    maskc = nc.dram_tensor("maskc", [P, 512], BF16, kind="ExternalInput").ap()
    maskr = nc.dram_tensor("maskr", [8, P], BF16, kind="ExternalInput").ap()
    y = nc.dram_tensor("y", [NT, DIM], F32, kind="ExternalOutput").ap()

    # Grouped token order: token (nb, a, mb, e) -> group index (nb, mb, a, e).
    xg = x.rearrange("(nb a mb e) f -> nb mb a e f", nb=2, a=4, mb=128, e=4)
    yg = y.rearrange("(nb a mb e) f -> nb mb a e f", nb=2, a=4, mb=128, e=4)

    with TileKernel(nc) as tc:
        ctx = tc._ctx
        const = ctx.enter_context(tc.tile_pool(name="const", bufs=1))
        xt_pool = ctx.enter_context(tc.tile_pool(name="xt", bufs=1))
        xload = ctx.enter_context(tc.tile_pool(name="xload", bufs=4))
        xbpool = ctx.enter_context(tc.tile_pool(name="xb", bufs=6))
        qk_pool = ctx.enter_context(tc.tile_pool(name="qk", bufs=3))
        v_pool = ctx.enter_context(tc.tile_pool(name="v", bufs=3))
        p_pool = ctx.enter_context(tc.tile_pool(name="p", bufs=4))
        r_pool = ctx.enter_context(tc.tile_pool(name="r", bufs=8))
        o_pool = ctx.enter_context(tc.tile_pool(name="o", bufs=3))
        ot_pool = ctx.enter_context(tc.tile_pool(name="ot", bufs=3))
        f_pool = ctx.enter_context(tc.tile_pool(name="f", bufs=4))
        psum = ctx.enter_context(tc.tile_pool(name="psum", bufs=2, space="PSUM"))

        # --- constants / weights ---
        wqk = []
        wv = []
        wo = []
        for c in range(4):
            t = const.tile([P, 1024], DT_MM, tag=f"wqk{c}")
            nc.sync.dma_start(t[:], wqkvT[c * P:(c + 1) * P, 0:1024])
            wqk.append(t)
            t = const.tile([P, 512], DT_MM, tag=f"wv{c}")
            nc.sync.dma_start(t[:], wqkvT[c * P:(c + 1) * P, 1024:1536])
            wv.append(t)
            t = const.tile([P, 512], DT_MM, tag=f"wo{c}")
            nc.sync.dma_start(t[:], woutT[c * P:(c + 1) * P, :])
            wo.append(t)
        ident = const.tile([P, P], F32, tag="ident")
        nc.sync.dma_start(ident[:], identc[:])
        identb = const.tile([P, P], BF16, tag="identb")
        nc.vector.tensor_copy(identb[:], ident[:])
        maskt = const.tile([P, 512], BF16, tag="maskt")
        nc.sync.dma_start(maskt[:], maskc[:])
        bias = const.tile([P, DIM], F32, tag="bias")
        nc.sync.dma_start(bias[:], biasb[:])

        # --- phase 0: load x (natural order), transpose, evict into xT in
        # grouped token order (permutation folded into the evict's free AP) ---
        xT = xt_pool.tile([P, 4 * NT], DT_MM)  # chunk c at cols [c*NT, ...)
        xTv = xT[:].rearrange("p (c nb mb a e) -> p c nb mb a e",
                              c=4, nb=2, mb=128, a=4, e=4)
        xbs = []
        for r in range(8):  # natural token row (nb*4 + a), 512 tokens each
            nb, a = divmod(r, 4)
            for tt in range(4):
                xb_t = xbpool.tile([P, DIM], BF16)
                nc.sync.dma_start(
                    xb_t[:], x[r * 512 + tt * P: r * 512 + (tt + 1) * P, :])
                xbs.append(xb_t)
            xbt4 = xbs[-4:]
            for c in range(4):
                ps = psum.tile([P, 512], F32, name="ps_xt", tag="mm", bufs=2)
                for tt in range(4):
                    nc.tensor.matmul(
                        ps[:, tt * P:(tt + 1) * P],
                        xbt4[tt][:, c * P:(c + 1) * P],
                        identb[:],
                        start=True, stop=True,
                    )
                # natural col (mb, e) of row (nb, a) -> grouped col
                # nb*2048 + mb*16 + a*4 + e
                dst = xTv[:, c, nb, :, a, :]
                src2 = ps[:].rearrange("p (mb e) -> p mb e", mb=128)
                if c % 2 == 0:
                    nc.vector.tensor_copy(dst, src2)
                else:
                    nc.scalar.copy(dst, src2)

        def emit_g2(gT, g_ot):
            for tt in range(4):
                ps = psum.tile([P, 512], F32, name="ps_g2", tag="mm", bufs=2)
                for c in range(4):
                    nc.tensor.matmul(
                        ps[:],
                        g_ot[:, c * 512 + tt * P: c * 512 + (tt + 1) * P],
                        wo[c][:],
                        start=(c == 0),
                        stop=(c == 3),
                    )
                fin = f_pool.tile([P, DIM], F32, name="fin")
                nc.vector.tensor_tensor(
                    fin[:], ps[:], bias[:], op=mybir.AluOpType.add)
                t_idx = gT * 4 + tt
                nb, ms = t_idx // 16, 8 * (t_idx % 16)
                for m in range(8):
                    nc.sync.dma_start(
                        yg[nb, ms + m], fin[16 * m:16 * (m + 1), :])

        prev_ot = None
        # --- main loop over 512-token supertiles ---
        for T in range(NSUP):
            tok = slice(T * 512, (T + 1) * 512)

            # GEMM1a: qkT [1024_fo, 512_tok] -> bf16
            qk_sb = qk_pool.tile([P, 8 * 512], BF16)
            for F in range(8):
                ps = psum.tile([P, 512], F32, name="ps_g1a", tag="mm", bufs=2)
                for c in range(4):
                    nc.tensor.matmul(
                        ps[:],
                        wqk[c][:, F * P:(F + 1) * P],
                        xT[:, c * NT + T * 512: c * NT + (T + 1) * 512],
                        start=(c == 0),
                        stop=(c == 3),
                    )
                dst = qk_sb[:, F * 512:(F + 1) * 512]
                if F % 2 == 0:
                    nc.vector.tensor_copy(dst, ps[:])
                else:
                    nc.scalar.copy(dst, ps[:])

            # GEMM1b: v [512_tok, 512_fo] -> bf16 (token-major), laid out with a
            # ones-column per head: v_sb cols per tt = [8 heads x (64 v | 1)]
            v_sb = v_pool.tile([P, 4 * 528], BF16)
            nc.vector.memset(v_sb[:], 1.0)
            for tt in range(4):
                ps = psum.tile([P, 512], F32, name="ps_g1b", tag="mm", bufs=2)
                for c in range(4):
                    nc.tensor.matmul(
                        ps[:],
                        xT[:, c * NT + T * 512 + tt * P:
                           c * NT + T * 512 + (tt + 1) * P],
                        wv[c][:],
                        start=(c == 0),
                        stop=(c == 3),
                    )
                vdst = v_sb[:, tt * 528: tt * 528 + 528].rearrange(
                    "p (h e) -> p h e", h=8)[:, :, 0:64]
                nc.vector.tensor_copy(
                    vdst, ps[:].rearrange("p (h d) -> p h d", h=8))


            # attention: tt outer, head-groups of 4 inner
            o_sb = o_pool.tile([P, 4 * 512], O_DT)
            for tt in range(4):
                psO = [
                    psum.tile([P, 288], F32, name=f"psO{half}", tag=f"psO{half}")
                    for half in range(2)
                ]
                for hg in range(2):
                    psS = psum.tile([P, 512], F32, name="psS", tag="psS", bufs=2)
                    for hh in range(4):
                        h = hg * 4 + hh
                        row = 64 * (h % 2)
                        qcol = (h // 2) * 512
                        kcol = (4 + h // 2) * 512
                        ksl = qk_sb[row:row + 64,
                                    kcol + tt * P: kcol + (tt + 1) * P]
                        qsl = qk_sb[row:row + 64,
                                    qcol + tt * P: qcol + (tt + 1) * P]
                        nc.tensor.matmul(
                            psS[:, hh * P:(hh + 1) * P], ksl, qsl,
                            start=True, stop=(MASKMUL == "dve"),
                        )
                        if MASKMUL == "mm":
                            nc.tensor.matmul(
                                psS[:, hh * P:(hh + 1) * P],
                                maskrt[:8, :], maskrt[:8, :],
                                start=False, stop=True,
                            )
                    p_sb = p_pool.tile([P, 512], BF16)
                    if MASKMUL == "mm":
                        nc.scalar.activation(
                            p_sb[:], psS[:], mybir.ActivationFunctionType.Exp,
                            bias=expb[:], scale=SCALE,
                        )
                    else:
                        p_raw = p_pool.tile(
                            [P, 512], BF16, name="p_raw", tag="praw")
                        if os.environ.get("KERNEL_SKIPMASK") == "1":
                            kwb = (dict(bias=zerob[:])
                                   if os.environ.get("KERNEL_EXPB") == "1"
                                   else {})
                            nc.scalar.activation(
                                p_sb[:], psS[:],
                                mybir.ActivationFunctionType.Exp, scale=SCALE,
                                **kwb,
                            )
                        else:
                            nc.scalar.activation(
                                p_raw[:], psS[:],
                                mybir.ActivationFunctionType.Exp, scale=SCALE,
                            )
                            nc.vector.tensor_tensor(
                                p_sb[:], p_raw[:], maskt[:],
                                op=mybir.AluOpType.mult)
                    for hh in range(4):
                        h = hg * 4 + hh
                        psl = p_sb[:, hh * P:(hh + 1) * P]
                        nc.tensor.matmul(
                            psO[hg][:, hh * 72: hh * 72 + 65],
                            psl,
                            v_sb[:, tt * 528 + h * 66: tt * 528 + h * 66 + 65],
                            start=True, stop=True,
                        )

                # normalize + evict O (token-major fp32)
                for hg in range(2):
                    rc = r_pool.tile([P, 4], F32)
                    dsl = psO[hg][:].rearrange("p (h e) -> p h e", h=4)
                    nc.vector.reciprocal(rc[:], dsl[:, :, 64:65].squeeze(2))
                    # dsl: [p, 4 heads (stride 72), 72]; cols 64 = denom
                    dst = o_sb[:, tt * 512 + hg * 256:
                               tt * 512 + (hg + 1) * 256].rearrange(
                        "p (h d) -> p h d", h=4)
                    rbc = rc[:].unsqueeze(2).broadcast_to([P, 4, DH])
                    nc.vector.tensor_tensor(
                        dst, dsl[:, :, 0:64], rbc, op=mybir.AluOpType.mult)

            # O^T: [inner, 512_tok] -- regular bf16 matmul lhsT.T @ I
            # (transpose-mode doesn't count as PE-busy for the HAM clock
            # gate; a regular matmul keeps the PE warm)
            ot_sb = ot_pool.tile([P, 4 * 512], DT_MM)
            for c in range(4):
                ps = psum.tile([P, 512], F32, name="ps_ot", tag="mm", bufs=2)
                for tt in range(4):
                    nc.tensor.matmul(
                        ps[:, tt * P:(tt + 1) * P],
                        o_sb[:, tt * 512 + c * P: tt * 512 + (c + 1) * P],
                        identb[:],
                        start=True, stop=True,
                    )
                nc.scalar.copy(ot_sb[:, c * 512:(c + 1) * 512], ps[:])

            if T > 0:
                emit_g2(T - 1, prev_ot)
            prev_ot = ot_sb

        emit_g2(NSUP - 1, prev_ot)

    nc.compile()
    return nc


class TileKernel:
    """TileContext wrapper that also owns an ExitStack for pools."""

    def __init__(self, nc):
        self.nc = nc
        self._tc = tile.TileContext(nc)
        self._ctx = ExitStack()

    def __enter__(self):
        tc = self._tc.__enter__()
        tc._ctx = self._ctx
        return tc

    def __exit__(self, *exc):
        self._ctx.close()
        return self._tc.__exit__(*exc)


def _host_inputs(x, w_qkv, w_out, b_out):
    wqkvT = np.ascontiguousarray(w_qkv.T).astype(NP_MM)
    woutT = np.ascontiguousarray(w_out.T).astype(NP_MM)
    biasb = np.ascontiguousarray(
        np.broadcast_to(b_out.astype(np.float32), (P, DIM)))
    identc = np.eye(P, dtype=np.float32)
    mask1 = np.kron(np.eye(8, dtype=np.float32), np.ones((16, 16), np.float32))
    mask = np.tile(mask1, (1, 4)).astype(ml_dtypes.bfloat16)
    maskr_np = np.zeros((8, P), dtype=ml_dtypes.bfloat16)
    for j in range(8):
        maskr_np[j, j * 16:(j + 1) * 16] = 16.0
    shared = {
        "wqkvT": wqkvT, "woutT": woutT, "biasb": biasb,
        "identc": identc, "maskc": mask, "maskr": maskr_np,
    }
    return [
        {"x": np.ascontiguousarray(x[i]).astype(ml_dtypes.bfloat16), **shared}
        for i in range(N_CORES)
    ]


_NC_CACHE = {}


def _get_nc():
    if "nc" not in _NC_CACHE:
        _NC_CACHE["nc"] = build_kernel()
    return _NC_CACHE["nc"]


def kernel(x, w_qkv, w_out, b_out, _trace=False, _trace_kwargs=None):
    nc = _get_nc()
    in_maps = _host_inputs(x, w_qkv, w_out, b_out)
    kw = {}
    if _trace:
        kw = dict(trace=True, **(_trace_kwargs or {}))
    res = run_bass_kernel_spmd(nc, in_maps, core_ids=list(range(N_CORES)), **kw)
    out = np.stack([res.results[i]["y"] for i in range(N_CORES)], axis=0)
    if _trace:
        kernel.last_results = res
    return out


# revision 56
# speedup vs baseline: 1.1063x; 1.1063x over previous
"""DiagBlockAttention Trainium2 kernel.

Full module: qkv = x @ w_qkv.T; block-diagonal attention over 16-token
groups (4x4 tiles of the (8, 512) token grid); out = attn_out @ w_out.T + b_out.

Sharding: data-parallel over batch -- batch element i runs on NeuronCore i
(no collectives).  All heavy matmuls run in bf16 on the TensorEngine with
fp32 PSUM accumulation (rel err vs fp32 oracle ~4e-3).

Per-core dataflow (x_b: [4096, 512] fp32):
  1. x -> SBUF, transpose to xT [512_fi, 4096_tok] via regular bf16
     identity matmuls (transpose-mode doesn't count as PE-busy for the HAM
     clock gate, regular matmuls do).  The 4x4-block token permutation that
     makes each 16-token attention group contiguous is folded into the
     free-dim access pattern of the PSUM->SBUF eviction.
  2. GEMM1a: qkT = W_qk-chunks.T @ xT -> feature-major q/k [1024_fo, tok].
     GEMM1b: v = xT-chunks.T @ W_v -> token-major v [tok, 512], stored with
     a ones-column per head (fused softmax denominator).
  3. Attention per (128-token tile, head): S^T = k^T.T @ q^T into PSUM
     [k, q], plus a rank-8 mask matmul (indicator rows valued 16) that adds
     +256 to same-group score entries; P = exp(S/8 - 32) on ScalarE (off-
     group entries -> e^-32 ~ 0, so the dense softmax equals the group-
     restricted softmax; the uniform in-group offset cancels).  One matmul
     per head computes [O' | denom] = P.T @ [v | 1]; normalize+evict via a
     broadcasted reciprocal multiply on VectorE.
  4. O -> O^T via regular bf16 identity matmuls; GEMM2 (pipelined one
     iteration behind): final = O^T-chunks.T @ W_out + b_out; stores
     un-permute back to natural token order.

Hardware notes baked into the structure (found by bisection on trn2):
  - a PSUM accumulation group whose matmuls interleave with another open
    group, or certain K=64 single-matmul groups at base_partition 64,
    fault the exec unit -> groups are kept strictly sequential and closed
    by a base-0 matmul (the mask matmul closes each S group);
  - LDWEIGHTS does not overlap its paired MATMUL on this toolchain
    (walrus --enable-ldw-opt is broken), so per-matmul cost ~ LDW + N.
"""

import os
import sys
from contextlib import ExitStack

sys.path.insert(0, "/opt/trn_rl_repo")

import ml_dtypes
import numpy as np

import concourse.bass as bass
import concourse.mybir as mybir
import concourse.tile as tile
from concourse import bacc
from concourse.bass_utils import run_bass_kernel_spmd


def _ensure_ntff_hook():
    """This image's antenv lacks axon_hooks; synthesize it so trace=True
    (NTFF profiling) works through run_bass_kernel_spmd."""
    import types

    try:
        from antenv import axon_hooks  # noqa: F401
        return
    except ImportError:
        pass
    try:
        import antenv
        from trn_agent_boot.trn_boot import _ntff_profile_via_ctypes

        mod = types.ModuleType("antenv.axon_hooks")
        _hook = [None]
        mod.set_axon_ntff_profile_hook = lambda h: _hook.__setitem__(0, h)
        mod.get_axon_ntff_profile_hook = lambda: _hook[0]
        sys.modules["antenv.axon_hooks"] = mod
        antenv.axon_hooks = mod
        mod.set_axon_ntff_profile_hook(
            _ntff_profile_via_ctypes("/opt/axon/libaxon_pjrt.so"))
    except Exception as e:  # pragma: no cover
        print(f"ntff hook shim failed ({e}); tracing disabled", file=sys.stderr)


_ensure_ntff_hook()

if os.environ.get("KERNEL_LDWOPT") == "1":
    from concourse import bass_utils as _bu

    _orig_run_command = _bu.run_command

    def _run_command_ldwopt(cmd, *a, **kw):
        cmd = ["--enable-ldw-opt=true" if c == "--enable-ldw-opt=false" else c
               for c in cmd]
        return _orig_run_command(cmd, *a, **kw)

    _bu.run_command = _run_command_ldwopt

F32 = mybir.dt.float32
F32R = mybir.dt.float32r
BF16 = mybir.dt.bfloat16

N_CORES = 8
NT = 4096          # tokens per core
DIM = 512          # model dim
INNER = 512        # heads * dim_head
HEADS = 8
DH = 64            # dim head
NSUP = NT // 512   # 512-token supertiles
P = 128

SCALE = DH ** -0.5

# Matmul compute dtype for the three big GEMMs: "fp32r" | "bf16" | "fp32"
MM_MODE = os.environ.get("KERNEL_MM_MODE", "bf16")
DT_MM = {"fp32r": F32R, "bf16": BF16, "fp32": F32}[MM_MODE]
NP_MM = {"fp32r": np.float32, "bf16": ml_dtypes.bfloat16,
         "fp32": np.float32}[MM_MODE]
O_DT = BF16


def build_kernel():
    nc = bacc.Bacc("TRN2", target_bir_lowering=False, debug=False)

    x = nc.dram_tensor("x", [NT, DIM], BF16, kind="ExternalInput").ap()
    wqkvT = nc.dram_tensor(
        "wqkvT", [DIM, 3 * INNER], DT_MM, kind="ExternalInput").ap()
    woutT = nc.dram_tensor("woutT", [INNER, DIM], DT_MM, kind="ExternalInput").ap()
    biasb = nc.dram_tensor("biasb", [P, DIM], F32, kind="ExternalInput").ap()
    identc = nc.dram_tensor("identc", [P, P], F32, kind="ExternalInput").ap()
    maskc = nc.dram_tensor("maskc", [P, 512], BF16, kind="ExternalInput").ap()
    maskr = nc.dram_tensor("maskr", [8, P], BF16, kind="ExternalInput").ap()
    y = nc.dram_tensor("y", [NT, DIM], F32, kind="ExternalOutput").ap()

    # Grouped token order: token (nb, a, mb, e) -> group index (nb, mb, a, e).
    xg = x.rearrange("(nb a mb e) f -> nb mb a e f", nb=2, a=4, mb=128, e=4)
    yg = y.rearrange("(nb a mb e) f -> nb mb a e f", nb=2, a=4, mb=128, e=4)

    with TileKernel(nc) as tc:
        ctx = tc._ctx
        const = ctx.enter_context(tc.tile_pool(name="const", bufs=1))
        xt_pool = ctx.enter_context(tc.tile_pool(name="xt", bufs=1))
        xload = ctx.enter_context(tc.tile_pool(name="xload", bufs=4))
        xbpool = ctx.enter_context(tc.tile_pool(name="xb", bufs=6))
        qk_pool = ctx.enter_context(tc.tile_pool(name="qk", bufs=4))
        v_pool = ctx.enter_context(tc.tile_pool(name="v", bufs=4))
        p_pool = ctx.enter_context(tc.tile_pool(name="p", bufs=5))
        r_pool = ctx.enter_context(tc.tile_pool(name="r", bufs=8))
        o_pool = ctx.enter_context(tc.tile_pool(name="o", bufs=3))
        ot_pool = ctx.enter_context(tc.tile_pool(name="ot", bufs=3))
        f_pool = ctx.enter_context(tc.tile_pool(name="f", bufs=4))
        psum = ctx.enter_context(tc.tile_pool(name="psum", bufs=2, space="PSUM"))

        # --- constants / weights ---
        wqk = []
        wv = []
        wo = []
        for c in range(4):
            t = const.tile([P, 1024], DT_MM, tag=f"wqk{c}")
            nc.sync.dma_start(t[:], wqkvT[c * P:(c + 1) * P, 0:1024])
            wqk.append(t)
            t = const.tile([P, 512], DT_MM, tag=f"wv{c}")
            nc.sync.dma_start(t[:], wqkvT[c * P:(c + 1) * P, 1024:1536])
            wv.append(t)
            t = const.tile([P, 512], DT_MM, tag=f"wo{c}")
            nc.sync.dma_start(t[:], woutT[c * P:(c + 1) * P, :])
            wo.append(t)
        ident = const.tile([P, P], F32, tag="ident")
        nc.sync.dma_start(ident[:], identc[:])
        identb = const.tile([P, P], BF16, tag="identb")
        nc.vector.tensor_copy(identb[:], ident[:])
        maskt = const.tile([P, 512], BF16, tag="maskt")
        nc.sync.dma_start(maskt[:], maskc[:])
        bias = const.tile([P, DIM], F32, tag="bias")
        nc.sync.dma_start(bias[:], biasb[:])

        # --- phase 0: load x (natural order), transpose, evict into xT in
        # grouped token order (permutation folded into the evict's free AP) ---
        xT = xt_pool.tile([P, 4 * NT], DT_MM)  # chunk c at cols [c*NT, ...)
        xTv = xT[:].rearrange("p (c nb mb a e) -> p c nb mb a e",
                              c=4, nb=2, mb=128, a=4, e=4)
        xbs = []
        for r in range(8):  # natural token row (nb*4 + a), 512 tokens each
            nb, a = divmod(r, 4)
            for tt in range(4):
                xb_t = xbpool.tile([P, DIM], BF16)
                nc.sync.dma_start(
                    xb_t[:], x[r * 512 + tt * P: r * 512 + (tt + 1) * P, :])
                xbs.append(xb_t)
            xbt4 = xbs[-4:]
            for c in range(4):
                ps = psum.tile([P, 512], F32, name="ps_xt", tag="mm", bufs=2)
                for tt in range(4):
                    nc.tensor.matmul(
                        ps[:, tt * P:(tt + 1) * P],
                        xbt4[tt][:, c * P:(c + 1) * P],
                        identb[:],
                        start=True, stop=True,
                    )
                # natural col (mb, e) of row (nb, a) -> grouped col
                # nb*2048 + mb*16 + a*4 + e
                dst = xTv[:, c, nb, :, a, :]
                src2 = ps[:].rearrange("p (mb e) -> p mb e", mb=128)
                if c % 2 == 0:
                    nc.vector.tensor_copy(dst, src2)
                else:
                    nc.scalar.copy(dst, src2)

        def emit_g2(gT, g_ot):
            for tt in range(4):
                ps = psum.tile([P, 512], F32, name="ps_g2", tag="mm", bufs=2)
                for c in range(4):
                    nc.tensor.matmul(
                        ps[:],
                        g_ot[:, c * 512 + tt * P: c * 512 + (tt + 1) * P],
                        wo[c][:],
                        start=(c == 0),
                        stop=(c == 3),
                    )
                fin = f_pool.tile([P, DIM], F32, name="fin")
                nc.vector.tensor_tensor(
                    fin[:], ps[:], bias[:], op=mybir.AluOpType.add)
                t_idx = gT * 4 + tt
                nb, ms = t_idx // 16, 8 * (t_idx % 16)
                for m in range(8):
                    nc.sync.dma_start(
                        yg[nb, ms + m], fin[16 * m:16 * (m + 1), :])

        prev_ot = None
        # --- main loop over 512-token supertiles ---
        for T in range(NSUP):
            tok = slice(T * 512, (T + 1) * 512)

            # GEMM1a: qkT [1024_fo, 512_tok] -> bf16
            qk_sb = qk_pool.tile([P, 8 * 512], BF16)
            for F in range(8):
                ps = psum.tile([P, 512], F32, name="ps_g1a", tag="mm", bufs=2)
                for c in range(4):
                    nc.tensor.matmul(
                        ps[:],
                        wqk[c][:, F * P:(F + 1) * P],
                        xT[:, c * NT + T * 512: c * NT + (T + 1) * 512],
                        start=(c == 0),
                        stop=(c == 3),
                    )
                dst = qk_sb[:, F * 512:(F + 1) * 512]
                if F % 2 == 0:
                    nc.vector.tensor_copy(dst, ps[:])
                else:
                    nc.scalar.copy(dst, ps[:])

            # GEMM1b: v [512_tok, 512_fo] -> bf16 (token-major), laid out with a
            # ones-column per head: v_sb cols per tt = [8 heads x (64 v | 1)]
            v_sb = v_pool.tile([P, 4 * 528], BF16)
            nc.vector.memset(v_sb[:], 1.0)
            for tt in range(4):
                ps = psum.tile([P, 512], F32, name="ps_g1b", tag="mm", bufs=2)
                for c in range(4):
                    nc.tensor.matmul(
                        ps[:],
                        xT[:, c * NT + T * 512 + tt * P:
                           c * NT + T * 512 + (tt + 1) * P],
                        wv[c][:],
                        start=(c == 0),
                        stop=(c == 3),
                    )
                vdst = v_sb[:, tt * 528: tt * 528 + 528].rearrange(
                    "p (h e) -> p h e", h=8)[:, :, 0:64]
                nc.vector.tensor_copy(
                    vdst, ps[:].rearrange("p (h d) -> p h d", h=8))


            # attention: tt outer, head-groups of 4 inner
            o_sb = o_pool.tile([P, 4 * 512], O_DT)
            for tt in range(4):
                psO = [
                    psum.tile([P, 288], F32, name=f"psO{half}", tag=f"psO{half}")
                    for half in range(2)
                ]
                for hg in range(2):
                    psS = psum.tile([P, 512], F32, name="psS", tag="psS", bufs=3)
                    for hh in range(4):
                        h = hg * 4 + hh
                        row = 64 * (h % 2)
                        qcol = (h // 2) * 512
                        kcol = (4 + h // 2) * 512
                        ksl = qk_sb[row:row + 64,
                                    kcol + tt * P: kcol + (tt + 1) * P]
                        qsl = qk_sb[row:row + 64,
                                    qcol + tt * P: qcol + (tt + 1) * P]
                        nc.tensor.matmul(
                            psS[:, hh * P:(hh + 1) * P], ksl, qsl,
                            start=True, stop=(MASKMUL == "dve"),
                        )
                        if MASKMUL == "mm":
                            nc.tensor.matmul(
                                psS[:, hh * P:(hh + 1) * P],
                                maskrt[:8, :], maskrt[:8, :],
                                start=False, stop=True,
                            )
                    p_sb = p_pool.tile([P, 512], BF16)
                    if MASKMUL == "mm":
                        nc.scalar.activation(
                            p_sb[:], psS[:], mybir.ActivationFunctionType.Exp,
                            bias=expb[:], scale=SCALE,
                        )
                    else:
                        p_raw = p_pool.tile(
                            [P, 512], BF16, name="p_raw", tag="praw")
                        if os.environ.get("KERNEL_SKIPMASK") == "1":
                            kwb = (dict(bias=zerob[:])
                                   if os.environ.get("KERNEL_EXPB") == "1"
                                   else {})
                            nc.scalar.activation(
                                p_sb[:], psS[:],
                                mybir.ActivationFunctionType.Exp, scale=SCALE,
                                **kwb,
                            )
                        else:
                            nc.scalar.activation(
                                p_raw[:], psS[:],
                                mybir.ActivationFunctionType.Exp, scale=SCALE,
                            )
                            nc.vector.tensor_tensor(
                                p_sb[:], p_raw[:], maskt[:],
                                op=mybir.AluOpType.mult)
                    for hh in range(4):
                        h = hg * 4 + hh
                        psl = p_sb[:, hh * P:(hh + 1) * P]
                        nc.tensor.matmul(
                            psO[hg][:, hh * 72: hh * 72 + 65],
                            psl,
                            v_sb[:, tt * 528 + h * 66: tt * 528 + h * 66 + 65],
                            start=True, stop=True,
                        )

                # normalize + evict O (token-major fp32)
                for hg in range(2):
                    rc = r_pool.tile([P, 4], F32)
                    dsl = psO[hg][:].rearrange("p (h e) -> p h e", h=4)
                    nc.vector.reciprocal(rc[:], dsl[:, :, 64:65].squeeze(2))
                    # dsl: [p, 4 heads (stride 72), 72]; cols 64 = denom
                    dst = o_sb[:, tt * 512 + hg * 256:
                               tt * 512 + (hg + 1) * 256].rearrange(
                        "p (h d) -> p h d", h=4)
                    rbc = rc[:].unsqueeze(2).broadcast_to([P, 4, DH])
                    nc.vector.tensor_tensor(
                        dst, dsl[:, :, 0:64], rbc, op=mybir.AluOpType.mult)

            # O^T: [inner, 512_tok] -- regular bf16 matmul lhsT.T @ I
            # (transpose-mode doesn't count as PE-busy for the HAM clock
            # gate; a regular matmul keeps the PE warm)
            ot_sb = ot_pool.tile([P, 4 * 512], DT_MM)
            for c in range(4):
                ps = psum.tile([P, 512], F32, name="ps_ot", tag="mm", bufs=2)
                for tt in range(4):
                    nc.tensor.matmul(
                        ps[:, tt * P:(tt + 1) * P],
                        o_sb[:, tt * 512 + c * P: tt * 512 + (c + 1) * P],
                        identb[:],
                        start=True, stop=True,
                    )
                nc.scalar.copy(ot_sb[:, c * 512:(c + 1) * 512], ps[:])

            if T > 0:
                emit_g2(T - 1, prev_ot)
            prev_ot = ot_sb

        emit_g2(NSUP - 1, prev_ot)

    nc.compile()
    return nc


class TileKernel:
    """TileContext wrapper that also owns an ExitStack for pools."""

    def __init__(self, nc):
        self.nc = nc
        self._tc = tile.TileContext(nc)
        self._ctx = ExitStack()

    def __enter__(self):
        tc = self._tc.__enter__()
        tc._ctx = self._ctx
        return tc

    def __exit__(self, *exc):
        self._ctx.close()
        return self._tc.__exit__(*exc)


def _host_inputs(x, w_qkv, w_out, b_out):
    wqkvT = np.ascontiguousarray(w_qkv.T).astype(NP_MM)
    woutT = np.ascontiguousarray(w_out.T).astype(NP_MM)
    biasb = np.ascontiguousarray(
        np.broadcast_to(b_out.astype(np.float32), (P, DIM)))
    identc = np.eye(P, dtype=np.float32)
    mask1 = np.kron(np.eye(8, dtype=np.float32), np.ones((16, 16), np.float32))
    mask = np.tile(mask1, (1, 4)).astype(ml_dtypes.bfloat16)
    maskr_np = np.zeros((8, P), dtype=ml_dtypes.bfloat16)
    for j in range(8):
        maskr_np[j, j * 16:(j + 1) * 16] = 16.0
    shared = {
        "wqkvT": wqkvT, "woutT": woutT, "biasb": biasb,
        "identc": identc, "maskc": mask, "maskr": maskr_np,
    }
    return [
        {"x": np.ascontiguousarray(x[i]).astype(ml_dtypes.bfloat16), **shared}
        for i in range(N_CORES)
    ]


_NC_CACHE = {}


def _get_nc():
    if "nc" not in _NC_CACHE:
        _NC_CACHE["nc"] = build_kernel()
    return _NC_CACHE["nc"]


def kernel(x, w_qkv, w_out, b_out, _trace=False, _trace_kwargs=None):
    nc = _get_nc()
    in_maps = _host_inputs(x, w_qkv, w_out, b_out)
    kw = {}
    if _trace:
        kw = dict(trace=True, **(_trace_kwargs or {}))
    res = run_bass_kernel_spmd(nc, in_maps, core_ids=list(range(N_CORES)), **kw)
    out = np.stack([res.results[i]["y"] for i in range(N_CORES)], axis=0)
    if _trace:
        kernel.last_results = res
    return out
